# revision 4
# baseline (speedup 1.0000x reference)
"""Bahdanau additive attention on 8 Trainium2 NeuronCores.

reference:
    eh = enc @ W_h.T            [B,S,H]
    qs = q   @ W_s.T            [B,T,H]
    score[b,t,s] = sum_h v[h] * tanh(eh[b,s,h] + qs[b,t,h])
    score = where(mask, score, -inf); attn = softmax_s(score)
    ctx[b,t,:] = sum_s attn[b,t,s] * enc[b,s,:]

Sharding: data-parallel over batch B=8, one batch per NeuronCore.

Per-core device program (all layouts chosen so the H-contraction of the
score sits on the partition axis):
  - inputs arrive pre-transposed from host: encT/qT [H,S|T], whT/wsT [H,H]
  - PE: ehT[d,s] = whT.T @ encT, qsT[d,t] = wsT.T @ qT   (PSUM, fp32)
  - main loop over s: DVE adds qsT + ehT[:,s] (per-partition scalar add),
    ACT computes tanh on large batched tiles (fp16 out),
    PE reduces over d via per-s stationary matvec with v -> score[t,s] PSUM
  - softmax over s on the free axis (DVE reduce_max / ACT exp+accum /
    DVE reciprocal), normalization folded into the context epilogue
  - PE transposes attn, context matmul against enc natural layout
"""

import sys

try:
    import concourse.bass as bass  # noqa: F401
except ImportError:  # pragma: no cover
    sys.path.insert(0, "/opt/trn_rl_repo")

import numpy as np

import concourse.bass as bass
import concourse.bacc as bacc
import concourse.mybir as mybir
from concourse import tile

FP32 = mybir.dt.float32
FP16 = mybir.dt.float16

N_CORES = 8
H = 512
T_FULL = 256
S_FULL = 256


def build_program(T=T_FULL, S=S_FULL, n_cores=N_CORES, nrep=1, debug=False):
    """Build the per-core Bass program. T/S parametrized for cheap sim runs."""
    assert H % 128 == 0 and T % 128 == 0 and S % 128 == 0
    DC = H // 128          # contraction chunks for the score reduction
    TB = T // 128          # t blocks (partition blocks of the score)
    SB = S // 128          # s blocks
    G = 4                  # s values batched per tanh activation

    nc = bacc.Bacc("TRN2", target_bir_lowering=False, debug=debug,
                   num_devices=n_cores)

    encT_d = nc.dram_tensor("encT", [H, S], FP32, kind="ExternalInput")
    enc_d = nc.dram_tensor("enc", [S, H], FP32, kind="ExternalInput")
    qT_d = nc.dram_tensor("qT", [H, T], FP32, kind="ExternalInput")
    whT_d = nc.dram_tensor("whT", [H, H], FP32, kind="ExternalInput")
    wsT_d = nc.dram_tensor("wsT", [H, H], FP32, kind="ExternalInput")
    v_d = nc.dram_tensor("v16", [128, DC], FP16, kind="ExternalInput")
    pen_d = nc.dram_tensor("pen", [1, S], FP32, kind="ExternalInput")
    ones_d = nc.dram_tensor("ones", [1, 128], FP32, kind="ExternalInput")
    ident_d = nc.dram_tensor("ident", [128, 128], FP32, kind="ExternalInput")
    ctx_d = nc.dram_tensor("ctx", [T, H], FP32, kind="ExternalOutput")

    with tile.TileContext(nc) as tc:
        with (
            tc.tile_pool(name="const", bufs=1) as const_pool,
            tc.tile_pool(name="proj", bufs=1) as proj_pool,
            tc.tile_pool(name="xadd", bufs=3) as xadd_pool,
            tc.tile_pool(name="xtanh", bufs=3) as xtanh_pool,
            tc.tile_pool(name="post", bufs=1) as post_pool,
            tc.tile_pool(name="ppsum", bufs=2, space=bass.MemorySpace.PSUM) as ppsum,
            tc.tile_pool(name="spsum", bufs=1, space=bass.MemorySpace.PSUM) as spsum,
            tc.tile_pool(name="apsum", bufs=2, space=bass.MemorySpace.PSUM) as apsum,
            tc.tile_pool(name="cpsum", bufs=2, space=bass.MemorySpace.PSUM) as cpsum,
        ):
            # ---- load constants / inputs ----
            whT_sb = [const_pool.tile([128, H], FP32, name=f"whT{i}", tag=f"whT{i}") for i in range(4)]
            wsT_sb = [const_pool.tile([128, H], FP32, name=f"wsT{i}", tag=f"wsT{i}") for i in range(4)]
            encT_sb = [const_pool.tile([128, S], FP32, name=f"encT{i}", tag=f"encT{i}") for i in range(DC)]
            qT_sb = [const_pool.tile([128, T], FP32, name=f"qT{i}", tag=f"qT{i}") for i in range(DC)]
            enc_sb = [const_pool.tile([128, H], FP32, name=f"enc{i}", tag=f"enc{i}") for i in range(SB)]
            v_sb = const_pool.tile([128, DC], FP16, tag="v")
            pen_sb = const_pool.tile([1, S], FP32, tag="pen")
            ones_sb = const_pool.tile([1, 128], FP32, tag="ones")
            ident_sb = const_pool.tile([128, 128], FP32, tag="ident")

            for i in range(4):
                nc.sync.dma_start(whT_sb[i][:], whT_d[128 * i:128 * (i + 1), :])
                nc.sync.dma_start(wsT_sb[i][:], wsT_d[128 * i:128 * (i + 1), :])
            for i in range(DC):
                nc.sync.dma_start(encT_sb[i][:], encT_d[128 * i:128 * (i + 1), :])
                nc.sync.dma_start(qT_sb[i][:], qT_d[128 * i:128 * (i + 1), :])
            for i in range(SB):
                nc.sync.dma_start(enc_sb[i][:], enc_d[128 * i:128 * (i + 1), :])
            nc.sync.dma_start(v_sb[:], v_d[:])
            nc.sync.dma_start(pen_sb[:], pen_d[:])
            nc.sync.dma_start(ones_sb[:], ones_d[:])
            nc.sync.dma_start(ident_sb[:], ident_d[:])

            import contextlib

            def _rep_ctx():
                if nrep == 1:
                    return contextlib.nullcontext()
                return tc.For_i(0, nrep, 1)

            with _rep_ctx():
                # ---- projections: ehT[dc] = (W_h @ encT chunk), qsT likewise ----
                ehT_sb = [post_pool.tile([128, S], FP32, name=f"ehT{i}", tag=f"ehT{i}") for i in range(DC)]
                qsT_sb = [post_pool.tile([128, T], FP32, name=f"qsT{i}", tag=f"qsT{i}") for i in range(DC)]
                for dc in range(DC):
                    eh_ps = ppsum.tile([128, S], FP32, tag="proj_ps")
                    for hc in range(4):
                        nc.tensor.matmul(
                            eh_ps[:],
                            whT_sb[hc][:, 128 * dc:128 * (dc + 1)],
                            encT_sb[hc][:],
                            start=(hc == 0), stop=(hc == 3),
                        )
                    nc.vector.tensor_copy(ehT_sb[dc][:], eh_ps[:])
                    qs_ps = ppsum.tile([128, T], FP32, tag="proj_ps")
                    for hc in range(4):
                        nc.tensor.matmul(
                            qs_ps[:],
                            wsT_sb[hc][:, 128 * dc:128 * (dc + 1)],
                            qT_sb[hc][:],
                            start=(hc == 0), stop=(hc == 3),
                        )
                    nc.vector.tensor_copy(qsT_sb[dc][:], qs_ps[:])

                # ---- score: psum[t, s] accumulated column by column ----
                score_ps = [spsum.tile([128, S], FP32, name=f"score{tb}", tag=f"score{tb}")
                            for tb in range(TB)]
                for tb in range(TB):
                    # seed every column with the mask penalty (broadcast over t)
                    nc.tensor.matmul(
                        score_ps[tb][:], ones_sb[:], pen_sb[:],
                        start=True, stop=False, skip_group_check=True,
                    )

                for g in range(S // G):
                    xadd = xadd_pool.tile([128, G * DC * T], FP32, tag="xadd")
                    for si in range(G):
                        s = g * G + si
                        for dc in range(DC):
                            nc.vector.tensor_scalar_add(
                                xadd[:, (si * DC + dc) * T:(si * DC + dc + 1) * T],
                                qsT_sb[dc][:],
                                ehT_sb[dc][:, s:s + 1],
                            )
                    xtanh = xtanh_pool.tile([128, G * DC * T], FP16, tag="xtanh")
                    nc.scalar.activation(
                        xtanh[:], xadd[:], mybir.ActivationFunctionType.Tanh)
                    for si in range(G):
                        s = g * G + si
                        for dc in range(DC):
                            base = (si * DC + dc) * T
                            for tb in range(TB):
                                nc.tensor.matmul(
                                    score_ps[tb][:, s:s + 1],
                                    xtanh[:, base + 128 * tb:base + 128 * (tb + 1)],
                                    v_sb[:, dc:dc + 1],
                                    start=False, stop=(dc == DC - 1),
                                    skip_group_check=True,
                                )

                # ---- softmax over s (free axis) ----
                attn_sb = [post_pool.tile([128, S], FP32, name=f"attn{tb}", tag=f"attn{tb}")
                           for tb in range(TB)]
                rden = [post_pool.tile([128, 1], FP32, name=f"rden{tb}", tag=f"rden{tb}")
                        for tb in range(TB)]
                for tb in range(TB):
                    nmax = post_pool.tile([128, 1], FP32, name=f"nmax{tb}", tag=f"nmax{tb}")
                    nc.vector.reduce_max(
                        nmax[:], score_ps[tb][:],
                        axis=mybir.AxisListType.X, negate=True)
                    den = post_pool.tile([128, 1], FP32, name=f"den{tb}", tag=f"den{tb}")
                    nc.scalar.activation(
                        attn_sb[tb][:], score_ps[tb][:],
                        mybir.ActivationFunctionType.Exp,
                        bias=nmax[:], scale=1.0, accum_out=den[:])
                    nc.vector.reciprocal(rden[tb][:], den[:])

                # ---- transpose attn -> attnT ----
                attnT_sb = [post_pool.tile([128, T], FP32, name=f"attnT{sb}", tag=f"attnT{sb}")
                            for sb in range(SB)]
                for sb in range(SB):
                    at_ps = apsum.tile([128, T], FP32, tag="at_ps")
                    for tb in range(TB):
                        nc.tensor.transpose(
                            at_ps[:, 128 * tb:128 * (tb + 1)],
                            attn_sb[tb][:, 128 * sb:128 * (sb + 1)],
                            ident_sb[:],
                        )
                    nc.vector.tensor_copy(attnT_sb[sb][:], at_ps[:])

                # ---- context: ctx[t, :] = sum_s attn[t,s] enc[s,:] (scaled) ----
                for tb in range(TB):
                    ctx_ps = cpsum.tile([128, H], FP32, tag="ctx_ps")
                    for sb in range(SB):
                        nc.tensor.matmul(
                            ctx_ps[:],
                            attnT_sb[sb][:, 128 * tb:128 * (tb + 1)],
                            enc_sb[sb][:],
                            start=(sb == 0), stop=(sb == SB - 1),
                        )
                    ctx_sb = post_pool.tile([128, H], FP32, name=f"ctx{tb}", tag=f"ctx{tb}")
                    nc.scalar.activation(
                        ctx_sb[:], ctx_ps[:],
                        mybir.ActivationFunctionType.Identity,
                        scale=rden[tb][:])
                    nc.sync.dma_start(ctx_d[128 * tb:128 * (tb + 1), :], ctx_sb[:])

    nc.compile()
    return nc


def make_in_maps(encoder_outputs, query, mask, W_h, W_s, v, T=T_FULL, S=S_FULL):
    B = encoder_outputs.shape[0]
    whT = np.ascontiguousarray(W_h.T.astype(np.float32))
    wsT = np.ascontiguousarray(W_s.T.astype(np.float32))
    v16 = np.ascontiguousarray(
        v.astype(np.float32).reshape(H // 128, 128).T.astype(np.float16))
    ones = np.ones((1, 128), np.float32)
    ident = np.eye(128, dtype=np.float32)
    in_maps = []
    for b in range(B):
        enc_b = np.ascontiguousarray(encoder_outputs[b].astype(np.float32))
        q_b = query[b].astype(np.float32)
        pen = np.where(mask[b], 0.0, -1e30).astype(np.float32).reshape(1, S)
        in_maps.append({
            "encT": np.ascontiguousarray(enc_b.T),
            "enc": enc_b,
            "qT": np.ascontiguousarray(q_b.T),
            "whT": whT,
            "wsT": wsT,
            "v16": v16,
            "pen": pen,
            "ones": ones,
            "ident": ident,
        })
    return in_maps


_PROGRAM_CACHE = {}


def kernel(encoder_outputs, query, mask, W_h, W_s, v):
    from concourse.bass_utils import run_bass_kernel_spmd

    B = encoder_outputs.shape[0]
    assert B == N_CORES
    key = (T_FULL, S_FULL, N_CORES)
    if key not in _PROGRAM_CACHE:
        _PROGRAM_CACHE[key] = build_program()
    nc = _PROGRAM_CACHE[key]
    in_maps = make_in_maps(encoder_outputs, query, mask, W_h, W_s, v)
    res = run_bass_kernel_spmd(nc, in_maps, list(range(N_CORES)))
    out = np.stack([res.results[b]["ctx"] for b in range(B)], axis=0)
    return out.astype(np.float32)


# revision 19
# speedup vs baseline: 67.8306x; 67.8306x over previous
"""Bahdanau additive attention on 8 Trainium2 NeuronCores.

reference:
    eh = enc @ W_h.T            [B,S,H]
    qs = q   @ W_s.T            [B,T,H]
    score[b,t,s] = sum_h v[h] * tanh(eh[b,s,h] + qs[b,t,h])
    score = where(mask, score, -inf); attn = softmax_s(score)
    ctx[b,t,:] = sum_s attn[b,t,s] * enc[b,s,:]

Sharding: data-parallel over batch B=8, one batch per NeuronCore.

Per-core device program (all layouts chosen so the H-contraction of the
score sits on the partition axis):
  - inputs arrive pre-transposed from host: encT/qT [H,S|T], whT/wsT [H,H]
  - PE: ehT[d,s] = whT.T @ encT, qsT[d,t] = wsT.T @ qT   (PSUM, fp32)
  - main loop over s: DVE adds qsT + ehT[:,s] (per-partition scalar add),
    ACT computes tanh on large batched tiles (fp16 out),
    PE reduces over d via per-s stationary matvec with v -> score[t,s] PSUM
  - softmax over s on the free axis (DVE reduce_max / ACT exp+accum /
    DVE reciprocal), normalization folded into the context epilogue
  - PE transposes attn, context matmul against enc natural layout
"""

import sys

try:
    import concourse.bass as bass  # noqa: F401
except ImportError:  # pragma: no cover
    sys.path.insert(0, "/opt/trn_rl_repo")

import numpy as np

import concourse.bass as bass
import concourse.bacc as bacc
import concourse.mybir as mybir
from concourse import tile

FP32 = mybir.dt.float32
FP16 = mybir.dt.float16

N_CORES = 8
H = 512
T_FULL = 256
S_FULL = 256


def build_program(T=T_FULL, S=S_FULL, n_cores=N_CORES, nrep=1, debug=False, ablate=()):
    """Build the per-core Bass program. T/S parametrized for cheap sim runs."""
    assert H % 128 == 0 and T % 128 == 0 and S % 128 == 0
    DC = H // 128          # contraction chunks for the score reduction
    TB = T // 128          # t blocks (partition blocks of the score)
    SB = S // 128          # s blocks
    G = 32                 # s values batched per tanh activation

    nc = bacc.Bacc("TRN2", target_bir_lowering=False, debug=debug,
                   num_devices=n_cores)

    encT_d = nc.dram_tensor("encT", [H, S], FP32, kind="ExternalInput")
    enc_d = nc.dram_tensor("enc", [S, H], FP32, kind="ExternalInput")
    qT_d = nc.dram_tensor("qT", [H, T], FP32, kind="ExternalInput")
    whT_d = nc.dram_tensor("whT", [H, H], FP32, kind="ExternalInput")
    wsT_d = nc.dram_tensor("wsT", [H, H], FP32, kind="ExternalInput")
    v_d = nc.dram_tensor("v16", [128, DC], FP16, kind="ExternalInput")
    pen_d = nc.dram_tensor("pen", [1, S], FP32, kind="ExternalInput")
    ones_d = nc.dram_tensor("ones", [1, 128], FP32, kind="ExternalInput")
    ident_d = nc.dram_tensor("ident", [128, 128], FP32, kind="ExternalInput")
    ctx_d = nc.dram_tensor("ctx", [T, H], FP32, kind="ExternalOutput")

    with tile.TileContext(nc) as tc:
        with (
            tc.tile_pool(name="const", bufs=1) as const_pool,
            tc.tile_pool(name="proj", bufs=1) as proj_pool,
            tc.tile_pool(name="xadd", bufs=2) as xadd_pool,
            tc.tile_pool(name="xtanh", bufs=3) as xtanh_pool,
            tc.tile_pool(name="post", bufs=1) as post_pool,
            tc.tile_pool(name="ppsum", bufs=2, space=bass.MemorySpace.PSUM) as ppsum,
            tc.tile_pool(name="spsum", bufs=1, space=bass.MemorySpace.PSUM) as spsum,
            tc.tile_pool(name="apsum", bufs=2, space=bass.MemorySpace.PSUM) as apsum,
            tc.tile_pool(name="cpsum", bufs=2, space=bass.MemorySpace.PSUM) as cpsum,
        ):
            # ---- load constants / inputs (few big DMAs; critical first) ----
            whT_cat = const_pool.tile([128, 4 * H], FP32, name="whT_cat")
            wsT_cat = const_pool.tile([128, 4 * H], FP32, name="wsT_cat")
            encT_cat = const_pool.tile([128, DC * S], FP32, name="encT_cat")
            qT_cat = const_pool.tile([128, DC * T], FP32, name="qT_cat")
            enc_cat = const_pool.tile([128, SB * H], FP32, name="enc_cat")
            v_sb = const_pool.tile([128, DC], FP16, tag="v")
            pen_sb = const_pool.tile([1, S], FP32, tag="pen")
            ones_sb = const_pool.tile([1, 128], FP32, tag="ones")
            ident_sb = const_pool.tile([128, 128], FP32, tag="ident")

            def cat_load(dst, src_d, blocks, width):
                nc.sync.dma_start(
                    dst[:].rearrange("p (a j) -> p a j", a=blocks),
                    src_d.rearrange("(a p) j -> p a j", p=128),
                )

            def w_cols(dst, src_d, lo, hi):
                nc.sync.dma_start(
                    dst[:].rearrange("p (a j) -> p a j", a=4)[:, :, lo:hi],
                    src_d.rearrange("(a p) j -> p a j", p=128)[:, :, lo:hi],
                )

            w_cols(whT_cat, whT_d, 0, 128)
            cat_load(encT_cat, encT_d, DC, S)
            w_cols(wsT_cat, wsT_d, 0, 128)
            cat_load(qT_cat, qT_d, DC, T)
            w_cols(whT_cat, whT_d, 128, H)
            w_cols(wsT_cat, wsT_d, 128, H)
            nc.sync.dma_start(pen_sb[:], pen_d[:])
            nc.sync.dma_start(ones_sb[:], ones_d[:])
            nc.sync.dma_start(v_sb[:], v_d[:])
            cat_load(enc_cat, enc_d, SB, H)
            nc.sync.dma_start(ident_sb[:], ident_d[:])

            whT_sb = [whT_cat[:, H * i:H * (i + 1)] for i in range(4)]
            wsT_sb = [wsT_cat[:, H * i:H * (i + 1)] for i in range(4)]
            encT_sb = [encT_cat[:, S * i:S * (i + 1)] for i in range(DC)]
            qT_sb = [qT_cat[:, T * i:T * (i + 1)] for i in range(DC)]
            enc_sb = [enc_cat[:, H * i:H * (i + 1)] for i in range(SB)]

            import contextlib

            def _rep_ctx():
                if nrep == 1:
                    return contextlib.nullcontext()
                return tc.For_i(0, nrep, 1)

            with _rep_ctx():
                # ---- projections, emitted lazily (dc+1 prefetched during dc) ----
                ehT_sb = [post_pool.tile([128, S], FP32, name=f"ehT{i}", tag=f"ehT{i}") for i in range(DC)]
                qsT_sb = [post_pool.tile([128, T], FP16, name=f"qsT{i}", tag=f"qsT{i}") for i in range(DC)]

                def project(dc):
                    eh_ps = ppsum.tile([128, S], FP32, tag="proj_ps")
                    for hc in range(4):
                        nc.tensor.matmul(
                            eh_ps[:],
                            whT_sb[hc][:, 128 * dc:128 * (dc + 1)],
                            encT_sb[hc],
                            start=(hc == 0), stop=(hc == 3),
                        )
                    nc.vector.tensor_copy(ehT_sb[dc][:], eh_ps[:])
                    qs_ps = ppsum.tile([128, T], FP32, tag="proj_ps")
                    for hc in range(4):
                        nc.tensor.matmul(
                            qs_ps[:],
                            wsT_sb[hc][:, 128 * dc:128 * (dc + 1)],
                            qT_sb[hc],
                            start=(hc == 0), stop=(hc == 3),
                        )
                    nc.vector.tensor_copy(qsT_sb[dc][:], qs_ps[:])

                project(0)

                # ---- score: psum[t, s] accumulated column by column ----
                score_ps = [spsum.tile([128, S], FP32, name=f"score{tb}", tag=f"score{tb}")
                            for tb in range(TB)]
                for tb in range(TB):
                    # seed every column with the mask penalty (broadcast over t)
                    nc.tensor.matmul(
                        score_ps[tb][:], ones_sb[:], pen_sb[:],
                        start=True, stop=False, skip_group_check=True,
                    )

                FUSED0 = 16 if G >= 32 else 0

                def chunks_for(dc):
                    rem = S - (FUSED0 if dc == 0 else 0)
                    first = [G // 2, G // 2] if dc == 0 else []
                    last = [16, 8, 8] if (dc == DC - 1 and S >= 64) else []
                    mid_total = rem - sum(first) - sum(last)
                    assert mid_total >= 0
                    mids = [G] * (mid_total // G)
                    if mid_total % G:
                        mids.append(mid_total % G)
                    ws = first + mids + last
                    out, s0 = [], FUSED0 if dc == 0 else 0
                    for w in ws:
                        out.append((s0, w))
                        s0 += w
                    assert s0 == S
                    return out

                for dc in range(DC):
                    if dc == 0 and FUSED0:
                        # ACT-fused lead-in: tanh(qsT + ehT[:, s]) with no DVE
                        # dependency, so ACT starts while DVE builds its lead
                        xf = xtanh_pool.tile([128, FUSED0 * T], FP16, tag="xtanh")
                        for si in range(FUSED0):
                            nc.scalar.activation(
                                xf[:, si * T:(si + 1) * T], qsT_sb[0][:],
                                mybir.ActivationFunctionType.Tanh,
                                bias=ehT_sb[0][:, si:si + 1])
                        if "mm" not in ablate:
                            for si in range(FUSED0):
                                base = si * T
                                for tb in range(TB):
                                    nc.tensor.matmul(
                                        score_ps[tb][:, si:si + 1],
                                        xf[:, base + 128 * tb:base + 128 * (tb + 1)],
                                        v_sb[:, 0:1],
                                        start=False, stop=False,
                                        skip_group_check=True,
                                    )
                    for ci, (s0, w) in enumerate(chunks_for(dc)):
                        if ci == 1 and dc + 1 < DC:
                            project(dc + 1)
                        xadd = xadd_pool.tile([128, w * T], FP16, tag="xadd")
                        if "dve" not in ablate:
                            for si in range(w):
                                s = s0 + si
                                nc.vector.tensor_scalar_add(
                                    xadd[:, si * T:(si + 1) * T],
                                    qsT_sb[dc][:],
                                    ehT_sb[dc][:, s:s + 1],
                                )
                        xtanh = xtanh_pool.tile([128, w * T], FP16, tag="xtanh")
                        if "act" not in ablate:
                            if "dve" in ablate:
                                src_ap = (qsT_sb[dc][:]
                                          .rearrange("p (o t) -> p o t", o=1)
                                          .broadcast_to([128, w, T]))
                                nc.scalar.activation(
                                    xtanh[:].rearrange("p (o t) -> p o t", o=w),
                                    src_ap, mybir.ActivationFunctionType.Tanh)
                            else:
                                nc.scalar.activation(
                                    xtanh[:], xadd[:],
                                    mybir.ActivationFunctionType.Tanh)
                        if "mm" not in ablate:
                            for si in range(w):
                                s = s0 + si
                                base = si * T
                                for tb in range(TB):
                                    nc.tensor.matmul(
                                        score_ps[tb][:, s:s + 1],
                                        xtanh[:, base + 128 * tb:base + 128 * (tb + 1)],
                                        v_sb[:, dc:dc + 1],
                                        start=False, stop=(dc == DC - 1),
                                        skip_group_check=True,
                                    )

                # ---- softmax over s (free axis) ----
                attn_sb = [post_pool.tile([128, S], FP32, name=f"attn{tb}", tag=f"attn{tb}")
                           for tb in range(TB)]
                rden = [post_pool.tile([128, 1], FP32, name=f"rden{tb}", tag=f"rden{tb}")
                        for tb in range(TB)]
                for tb in range(TB):
                    nmax = post_pool.tile([128, 1], FP32, name=f"nmax{tb}", tag=f"nmax{tb}")
                    nc.vector.reduce_max(
                        nmax[:], score_ps[tb][:],
                        axis=mybir.AxisListType.X, negate=True)
                    den = post_pool.tile([128, 1], FP32, name=f"den{tb}", tag=f"den{tb}")
                    nc.scalar.activation(
                        attn_sb[tb][:], score_ps[tb][:],
                        mybir.ActivationFunctionType.Exp,
                        bias=nmax[:], scale=1.0, accum_out=den[:])
                    nc.vector.reciprocal(rden[tb][:], den[:])

                # ---- transpose attn -> attnT ----
                attnT_sb = [post_pool.tile([128, T], FP32, name=f"attnT{sb}", tag=f"attnT{sb}")
                            for sb in range(SB)]
                for sb in range(SB):
                    at_ps = apsum.tile([128, T], FP32, tag="at_ps")
                    for tb in range(TB):
                        nc.tensor.transpose(
                            at_ps[:, 128 * tb:128 * (tb + 1)],
                            attn_sb[tb][:, 128 * sb:128 * (sb + 1)],
                            ident_sb[:],
                        )
                    nc.vector.tensor_copy(attnT_sb[sb][:], at_ps[:])

                # ---- context: ctx[t, :] = sum_s attn[t,s] enc[s,:] (scaled) ----
                for tb in range(TB):
                    ctx_ps = cpsum.tile([128, H], FP32, tag="ctx_ps")
                    for sb in range(SB):
                        nc.tensor.matmul(
                            ctx_ps[:],
                            attnT_sb[sb][:, 128 * tb:128 * (tb + 1)],
                            enc_sb[sb],
                            start=(sb == 0), stop=(sb == SB - 1),
                        )
                    ctx_sb = post_pool.tile([128, H], FP32, name=f"ctx{tb}", tag=f"ctx{tb}")
                    nc.scalar.activation(
                        ctx_sb[:], ctx_ps[:],
                        mybir.ActivationFunctionType.Identity,
                        scale=rden[tb][:])
                    nc.sync.dma_start(ctx_d[128 * tb:128 * (tb + 1), :], ctx_sb[:])

    nc.compile()
    return nc


def make_in_maps(encoder_outputs, query, mask, W_h, W_s, v, T=T_FULL, S=S_FULL):
    B = encoder_outputs.shape[0]
    whT = np.ascontiguousarray(W_h.T.astype(np.float32))
    wsT = np.ascontiguousarray(W_s.T.astype(np.float32))
    v16 = np.ascontiguousarray(
        v.astype(np.float32).reshape(H // 128, 128).T.astype(np.float16))
    ones = np.ones((1, 128), np.float32)
    ident = np.eye(128, dtype=np.float32)
    in_maps = []
    for b in range(B):
        enc_b = np.ascontiguousarray(encoder_outputs[b].astype(np.float32))
        q_b = query[b].astype(np.float32)
        pen = np.where(mask[b], 0.0, -1e30).astype(np.float32).reshape(1, S)
        in_maps.append({
            "encT": np.ascontiguousarray(enc_b.T),
            "enc": enc_b,
            "qT": np.ascontiguousarray(q_b.T),
            "whT": whT,
            "wsT": wsT,
            "v16": v16,
            "pen": pen,
            "ones": ones,
            "ident": ident,
        })
    return in_maps


_PROGRAM_CACHE = {}


def kernel(encoder_outputs, query, mask, W_h, W_s, v):
    from concourse.bass_utils import run_bass_kernel_spmd

    B = encoder_outputs.shape[0]
    assert B == N_CORES
    key = (T_FULL, S_FULL, N_CORES)
    if key not in _PROGRAM_CACHE:
        _PROGRAM_CACHE[key] = build_program()
    nc = _PROGRAM_CACHE[key]
    in_maps = make_in_maps(encoder_outputs, query, mask, W_h, W_s, v)
    res = run_bass_kernel_spmd(nc, in_maps, list(range(N_CORES)))
    out = np.stack([res.results[b]["ctx"] for b in range(B)], axis=0)
    return out.astype(np.float32)
